# revision 3
# baseline (speedup 1.0000x reference)
"""Trainium2 Bass kernel: NeRF segmented cumulative product head.

Computes, for a flat stream of samples grouped into contiguous ray segments:
    v[i]     = (1 - alpha[i]) + 1e-11
    cumprod  = segmented cumulative product of v (reset at each ray start)
    t[i]     = cumprod[i-1]  (global shift, t[0] = 1)
    weights  = alpha * t
    alphainv_last[r] = cumprod at the last sample of ray r

Device algorithm: the segmented cumprod is computed by the VectorE
`tensor_tensor_scan` instruction with ops (max, mult):
    state' = max(S[i], state) * v[i]
where S is the 0/1 segment-start mask.  At a start, max(1, state) = 1
(running products never exceed 1 + O(1e-11)), so the state resets; else
max(0, state) = state and the product continues.  The scan output is
written one column to the right, so the tile directly holds the shifted
transmittance t, and weights = alpha * t is one more elementwise op.

Sharding: the flat stream is split into 1024 ray-aligned rows
(8 cores x 128 partitions), each padded to a fixed length F, so every
partition scans an independent chain — no cross-partition or cross-core
communication.  Host numpy does the ray-aligned layout (searchsorted +
masked scatter), the start-mask encoding, the 1024 row-start weight
fixups, and the per-ray alphainv gather.
"""

import numpy as np

import concourse.bacc as bacc
import concourse.mybir as mybir
import concourse.tile as tile
from concourse.bass_utils import run_bass_kernel_spmd

# ---- problem constants (hardcoded; kernel.py must be self-contained) ----
N = 16_777_216
N_RAY = 65_536
NCORES = 8
P = 128
ROWS = NCORES * P        # 1024 ray-aligned rows
F = 16_896               # padded row length  (16384 avg + 512 slack)
NB = 8                   # column blocks per row
PAD_ALPHA = np.float32(0.5)

_f32 = mybir.dt.float32
_u8 = mybir.dt.uint8


def _emit_body(tc, a_ap, m_ap, w_ap, f, nb):
    """Per-core body.  a_ap [128,f] f32 alpha, m_ap [128,f] u8 start mask,
    w_ap [128,f] f32 weights out."""
    nc = tc.nc
    fb = f // nb
    mult = mybir.AluOpType.mult
    add = mybir.AluOpType.add
    amax = mybir.AluOpType.max
    Copy = mybir.ActivationFunctionType.Copy

    with (
        tc.tile_pool(name="mask", bufs=1) as maskp,
        tc.tile_pool(name="ld", bufs=3) as ld,
        tc.tile_pool(name="mid", bufs=2) as mid,
        tc.tile_pool(name="scan", bufs=3) as scan_pool,
        tc.tile_pool(name="st", bufs=3) as st,
    ):
        mt = maskp.tile([P, f], _u8)
        nc.sync.dma_start(mt[:], m_ap[:, :])

        t_prev = None
        for j in range(nb):
            c0 = j * fb
            at = ld.tile([P, fb], _f32, tag="alpha")
            nc.sync.dma_start(at[:], a_ap[:, c0 : c0 + fb])

            # u = 1 - alpha   (ScalarE);  v = u + 1e-11   (VectorE, 2x mode)
            ut = mid.tile([P, fb], _f32, tag="u")
            nc.scalar.activation(ut[:], at[:], Copy, bias=1.0, scale=-1.0)
            vt = mid.tile([P, fb], _f32, tag="v")
            nc.vector.tensor_scalar(vt[:], ut[:], 1e-11, None, add)

            # transmittance tile: t[:,0] is the carry-in, scan writes t[:,1:]
            tt = scan_pool.tile([P, fb + 1], _f32, tag="t")
            if j == 0:
                nc.gpsimd.memset(tt[:, 0:1], 1.0)
            else:
                nc.scalar.copy(tt[:, 0:1], t_prev[:, fb : fb + 1])
            nc.vector.tensor_tensor_scan(
                tt[:, 1 : fb + 1],
                mt[:, c0 : c0 + fb],
                vt[:],
                tt[:, 0:1],
                amax,
                mult,
            )
            # w = alpha * t   (GpSimd)
            wt = st.tile([P, fb], _f32, tag="w")
            nc.gpsimd.tensor_tensor(wt[:], at[:], tt[:, 0:fb], mult)
            nc.sync.dma_start(w_ap[:, c0 : c0 + fb], wt[:])
            t_prev = tt


def build_module(f=F, nb=NB, n_cores=NCORES, repeat=1):
    nc = bacc.Bacc(
        "TRN2", target_bir_lowering=False, debug=False, num_devices=n_cores
    )
    a = nc.dram_tensor("alpha", [P, f], _f32, kind="ExternalInput")
    m = nc.dram_tensor("mask", [P, f], _u8, kind="ExternalInput")
    w = nc.dram_tensor("w", [P, f], _f32, kind="ExternalOutput")
    with tile.TileContext(nc) as tc:
        for _ in range(repeat):
            _emit_body(tc, a.ap(), m.ap(), w.ap(), f, nb)
    nc.compile()
    return nc


_MODULE = None


def _get_module():
    global _MODULE
    if _MODULE is None:
        _MODULE = build_module()
    return _MODULE


# --------------------------- host-side helpers ---------------------------


def shard_inputs(alpha, ray_id, rows=ROWS, f=F):
    """Split the flat stream into `rows` ray-aligned rows padded to length f.

    Returns (alpha2d [rows,f] f32, mask2d [rows,f] u8, bounds [rows+1],
    lens [rows], idx [rows,f] bool).  mask2d is 1 at segment starts (every
    row start is a ray start by construction); padding columns are 0 with
    pad alpha = PAD_ALPHA so the first pad column's weight recovers the
    row's final cumprod (the "carry").
    """
    n = alpha.shape[0]
    targets = (np.arange(1, rows) * (n // rows)).astype(np.int64)
    ray_at = ray_id[targets]
    s = np.searchsorted(ray_id, ray_at, side="left")
    bounds = np.concatenate([[0], s, [n]]).astype(np.int64)
    lens = np.diff(bounds)
    assert lens.min() >= 0 and lens.max() < f, (lens.min(), lens.max(), f)

    idx = np.arange(f)[None, :] < lens[:, None]           # [rows, f]
    alpha2d = np.full((rows, f), PAD_ALPHA, np.float32)
    alpha2d[idx] = alpha
    start_flat = np.empty(n, np.uint8)
    start_flat[0] = 1
    np.not_equal(ray_id[1:], ray_id[:-1], out=start_flat[1:].view(bool))
    mask2d = np.zeros((rows, f), np.uint8)
    mask2d[idx] = start_flat
    return alpha2d, mask2d, bounds, lens, idx


def postprocess(w2d, alpha, ray_id, n_ray, bounds, lens, idx):
    """Assemble full weights + alphainv_last from the padded device output."""
    rows = w2d.shape[0]
    n = alpha.shape[0]

    # carry[k] = cumprod at the last real element of row k
    #          = t at first pad column = w[k, lens[k]] / pad_alpha
    carry = w2d[np.arange(rows), lens] * np.float32(1.0 / PAD_ALPHA)
    empty_rows = lens == 0
    if empty_rows.any():        # forward-fill carries through empty rows
        ff = np.where(empty_rows, 0, np.arange(rows))
        np.maximum.accumulate(ff, out=ff)
        carry = carry[ff]
        carry[: int(np.argmax(~empty_rows))] = 1.0

    weights = np.empty(n, np.float32)
    weights[:] = w2d[idx]

    # alphainv_last[r] = cumprod at last sample of ray r.
    cnt_hi = np.searchsorted(ray_id, np.arange(n_ray), side="right")
    cnt_lo = np.searchsorted(ray_id, np.arange(n_ray), side="left")
    nonempty = cnt_hi > cnt_lo
    last = cnt_hi - 1                                     # valid where nonempty

    alphainv = np.empty(n_ray, np.float32)
    v0 = (np.float32(1.0) - alpha[0]) + np.float32(1e-11)
    alphainv[~nonempty] = v0                              # reference quirk

    j = last[nonempty]
    row_of = np.searchsorted(bounds, j, side="right") - 1
    is_row_last = j == bounds[row_of + 1] - 1
    vals = np.empty(j.shape[0], np.float32)
    vals[is_row_last] = carry[row_of[is_row_last]]
    jn = j[~is_row_last] + 1                              # next elem, same row
    a_n = alpha[jn]
    with np.errstate(divide="ignore", invalid="ignore"):
        vals[~is_row_last] = weights[jn] / a_n
    # exact-zero (or absurdly small) alpha at the probe point: recompute ray
    bad = np.zeros(j.shape[0], bool)
    bad[~is_row_last] = a_n < 1e-20
    if bad.any():
        ray_ids_ne = np.arange(n_ray)[nonempty]
        for k in np.where(bad)[0]:
            r = ray_ids_ne[k]
            seg = alpha[cnt_lo[r] : cnt_hi[r]]
            v = (np.float32(1.0) - seg) + np.float32(1e-11)
            p = np.float32(1.0)
            for x in v:
                p = np.float32(p * x)
            vals[k] = p
    alphainv[nonempty] = vals

    # row-start weight fixups: t at a row start is the previous row's carry
    for k in range(rows):
        if lens[k] > 0:
            i = bounds[k]
            t = carry[k - 1] if k > 0 else np.float32(1.0)
            weights[i] = np.float32(alpha[i] * t)

    return weights, alphainv


def kernel(alpha, ray_id, N_ray):
    alpha = np.asarray(alpha, np.float32).reshape(-1)
    ray_id = np.asarray(ray_id, np.int32).reshape(-1)
    n_ray = int(N_ray)
    assert alpha.shape[0] == N and n_ray == N_RAY

    alpha2d, mask2d, bounds, lens, idx = shard_inputs(alpha, ray_id)
    nc = _get_module()
    in_maps = [
        {
            "alpha": np.ascontiguousarray(alpha2d[c * P : (c + 1) * P]),
            "mask": np.ascontiguousarray(mask2d[c * P : (c + 1) * P]),
        }
        for c in range(NCORES)
    ]
    res = run_bass_kernel_spmd(nc, in_maps, core_ids=list(range(NCORES)))
    w2d = np.concatenate(
        [res.results[c]["w"] for c in range(NCORES)], axis=0
    ).astype(np.float32, copy=False)
    return postprocess(w2d, alpha, ray_id, n_ray, bounds, lens, idx)


# revision 4
# speedup vs baseline: 6.8342x; 6.8342x over previous
"""Trainium2 Bass kernel: NeRF segmented cumulative product head.

Computes, for a flat stream of samples grouped into contiguous ray segments:
    v[i]     = (1 - alpha[i]) + 1e-11
    cumprod  = segmented cumulative product of v (reset at each ray start)
    t[i]     = cumprod[i-1]  (global shift, t[0] = 1)
    weights  = alpha * t
    alphainv_last[r] = cumprod at the last sample of ray r

Device algorithm: the segmented cumprod is computed by the VectorE
`tensor_tensor_scan` instruction with ops (max, mult):
    state' = max(S[i], state) * v[i]
where S is the 0/1 segment-start mask.  At a start, max(1, state) = 1
(running products never exceed 1 + O(1e-11)), so the state resets; else
max(0, state) = state and the product continues.  The scan output is
written one column to the right, so the tile directly holds the shifted
transmittance t, and weights = alpha * t is one more elementwise op.

Sharding: the flat stream is split into 1024 ray-aligned rows
(8 cores x 128 partitions), each padded to a fixed length F, so every
partition scans an independent chain — no cross-partition or cross-core
communication.  Host numpy does the ray-aligned layout (searchsorted +
masked scatter), the start-mask encoding, the 1024 row-start weight
fixups, and the per-ray alphainv gather.
"""

import numpy as np

import concourse.bacc as bacc
import concourse.mybir as mybir
import concourse.tile as tile
from concourse.bass_utils import run_bass_kernel_spmd

# ---- problem constants (hardcoded; kernel.py must be self-contained) ----
N = 16_777_216
N_RAY = 65_536
NCORES = 8
P = 128
ROWS = NCORES * P        # 1024 ray-aligned rows
F = 16_896               # padded row length  (16384 avg + 512 slack)
NB = 8                   # column blocks per row
PAD_ALPHA = np.float32(0.5)

_f32 = mybir.dt.float32
_u8 = mybir.dt.uint8


def _emit_pass(tc, pools, a_ap, m_ap, w_ap, f, nb):
    """One full pass over the core's data.  a_ap [128,f] f32 alpha,
    m_ap [128,f] u8 start mask, w_ap [128,f] f32 weights out."""
    nc = tc.nc
    maskp, ld, mid, scan_pool, st = pools
    fb = f // nb
    mult = mybir.AluOpType.mult
    add = mybir.AluOpType.add
    amax = mybir.AluOpType.max
    Copy = mybir.ActivationFunctionType.Copy

    mt = maskp.tile([P, f], _u8, tag="mask")
    nc.sync.dma_start(mt[:], m_ap[:, :])

    t_prev = None
    for j in range(nb):
        c0 = j * fb
        at = ld.tile([P, fb], _f32, tag="alpha")
        nc.sync.dma_start(at[:], a_ap[:, c0 : c0 + fb])

        # u = 1 - alpha   (ScalarE);  v = u + 1e-11   (VectorE, 2x mode)
        ut = mid.tile([P, fb], _f32, tag="u")
        nc.scalar.activation(ut[:], at[:], Copy, bias=1.0, scale=-1.0)
        vt = mid.tile([P, fb], _f32, tag="v")
        nc.vector.tensor_scalar(vt[:], ut[:], 1e-11, None, add)

        # transmittance tile: t[:,0] is the carry-in, scan writes t[:,1:]
        tt = scan_pool.tile([P, fb + 1], _f32, tag="t")
        if j == 0:
            nc.gpsimd.memset(tt[:, 0:1], 1.0)
        else:
            nc.scalar.copy(tt[:, 0:1], t_prev[:, fb : fb + 1])
        nc.vector.tensor_tensor_scan(
            tt[:, 1 : fb + 1],
            mt[:, c0 : c0 + fb],
            vt[:],
            tt[:, 0:1],
            amax,
            mult,
        )
        # w = alpha * t   (GpSimd)
        wt = st.tile([P, fb], _f32, tag="w")
        nc.gpsimd.tensor_tensor(wt[:], at[:], tt[:, 0:fb], mult)
        nc.sync.dma_start(w_ap[:, c0 : c0 + fb], wt[:])
        t_prev = tt


def build_module(f=F, nb=NB, n_cores=NCORES, loop=1):
    nc = bacc.Bacc(
        "TRN2", target_bir_lowering=False, debug=False, num_devices=n_cores
    )
    a = nc.dram_tensor("alpha", [P, f], _f32, kind="ExternalInput")
    m = nc.dram_tensor("mask", [P, f], _u8, kind="ExternalInput")
    w = nc.dram_tensor("w", [P, f], _f32, kind="ExternalOutput")
    with tile.TileContext(nc) as tc:
        with (
            tc.tile_pool(name="mask", bufs=1) as maskp,
            tc.tile_pool(name="ld", bufs=3) as ld,
            tc.tile_pool(name="mid", bufs=2) as mid,
            tc.tile_pool(name="scan", bufs=3) as scan_pool,
            tc.tile_pool(name="st", bufs=3) as st,
        ):
            pools = (maskp, ld, mid, scan_pool, st)
            if loop > 1:
                with tc.For_i(0, loop, 1):
                    _emit_pass(tc, pools, a.ap(), m.ap(), w.ap(), f, nb)
            else:
                _emit_pass(tc, pools, a.ap(), m.ap(), w.ap(), f, nb)
    nc.compile()
    return nc


_MODULE = None


def _get_module():
    global _MODULE
    if _MODULE is None:
        _MODULE = build_module()
    return _MODULE


# --------------------------- host-side helpers ---------------------------


def shard_inputs(alpha, ray_id, rows=ROWS, f=F):
    """Split the flat stream into `rows` ray-aligned rows padded to length f.

    Returns (alpha2d [rows,f] f32, mask2d [rows,f] u8, bounds [rows+1],
    lens [rows], idx [rows,f] bool).  mask2d is 1 at segment starts (every
    row start is a ray start by construction); padding columns are 0 with
    pad alpha = PAD_ALPHA so the first pad column's weight recovers the
    row's final cumprod (the "carry").
    """
    n = alpha.shape[0]
    targets = (np.arange(1, rows) * (n // rows)).astype(np.int64)
    ray_at = ray_id[targets]
    s = np.searchsorted(ray_id, ray_at, side="left")
    bounds = np.concatenate([[0], s, [n]]).astype(np.int64)
    lens = np.diff(bounds)
    assert lens.min() >= 0 and lens.max() < f, (lens.min(), lens.max(), f)

    idx = np.arange(f)[None, :] < lens[:, None]           # [rows, f]
    alpha2d = np.full((rows, f), PAD_ALPHA, np.float32)
    alpha2d[idx] = alpha
    start_flat = np.empty(n, np.uint8)
    start_flat[0] = 1
    np.not_equal(ray_id[1:], ray_id[:-1], out=start_flat[1:].view(bool))
    mask2d = np.zeros((rows, f), np.uint8)
    mask2d[idx] = start_flat
    return alpha2d, mask2d, bounds, lens, idx


def postprocess(w2d, alpha, ray_id, n_ray, bounds, lens, idx):
    """Assemble full weights + alphainv_last from the padded device output."""
    rows = w2d.shape[0]
    n = alpha.shape[0]

    # carry[k] = cumprod at the last real element of row k
    #          = t at first pad column = w[k, lens[k]] / pad_alpha
    carry = w2d[np.arange(rows), lens] * np.float32(1.0 / PAD_ALPHA)
    empty_rows = lens == 0
    if empty_rows.any():        # forward-fill carries through empty rows
        ff = np.where(empty_rows, 0, np.arange(rows))
        np.maximum.accumulate(ff, out=ff)
        carry = carry[ff]
        carry[: int(np.argmax(~empty_rows))] = 1.0

    weights = np.empty(n, np.float32)
    weights[:] = w2d[idx]

    # alphainv_last[r] = cumprod at last sample of ray r.
    cnt_hi = np.searchsorted(ray_id, np.arange(n_ray), side="right")
    cnt_lo = np.searchsorted(ray_id, np.arange(n_ray), side="left")
    nonempty = cnt_hi > cnt_lo
    last = cnt_hi - 1                                     # valid where nonempty

    alphainv = np.empty(n_ray, np.float32)
    v0 = (np.float32(1.0) - alpha[0]) + np.float32(1e-11)
    alphainv[~nonempty] = v0                              # reference quirk

    j = last[nonempty]
    row_of = np.searchsorted(bounds, j, side="right") - 1
    is_row_last = j == bounds[row_of + 1] - 1
    vals = np.empty(j.shape[0], np.float32)
    vals[is_row_last] = carry[row_of[is_row_last]]
    jn = j[~is_row_last] + 1                              # next elem, same row
    a_n = alpha[jn]
    with np.errstate(divide="ignore", invalid="ignore"):
        vals[~is_row_last] = weights[jn] / a_n
    # exact-zero (or absurdly small) alpha at the probe point: recompute ray
    bad = np.zeros(j.shape[0], bool)
    bad[~is_row_last] = a_n < 1e-20
    if bad.any():
        ray_ids_ne = np.arange(n_ray)[nonempty]
        for k in np.where(bad)[0]:
            r = ray_ids_ne[k]
            seg = alpha[cnt_lo[r] : cnt_hi[r]]
            v = (np.float32(1.0) - seg) + np.float32(1e-11)
            p = np.float32(1.0)
            for x in v:
                p = np.float32(p * x)
            vals[k] = p
    alphainv[nonempty] = vals

    # row-start weight fixups: t at a row start is the previous row's carry
    for k in range(rows):
        if lens[k] > 0:
            i = bounds[k]
            t = carry[k - 1] if k > 0 else np.float32(1.0)
            weights[i] = np.float32(alpha[i] * t)

    return weights, alphainv


def kernel(alpha, ray_id, N_ray):
    alpha = np.asarray(alpha, np.float32).reshape(-1)
    ray_id = np.asarray(ray_id, np.int32).reshape(-1)
    n_ray = int(N_ray)
    assert alpha.shape[0] == N and n_ray == N_RAY

    alpha2d, mask2d, bounds, lens, idx = shard_inputs(alpha, ray_id)
    nc = _get_module()
    in_maps = [
        {
            "alpha": np.ascontiguousarray(alpha2d[c * P : (c + 1) * P]),
            "mask": np.ascontiguousarray(mask2d[c * P : (c + 1) * P]),
        }
        for c in range(NCORES)
    ]
    res = run_bass_kernel_spmd(nc, in_maps, core_ids=list(range(NCORES)))
    w2d = np.concatenate(
        [res.results[c]["w"] for c in range(NCORES)], axis=0
    ).astype(np.float32, copy=False)
    return postprocess(w2d, alpha, ray_id, n_ray, bounds, lens, idx)


# revision 8
# speedup vs baseline: 7.8547x; 1.1493x over previous
"""Trainium2 Bass kernel: NeRF segmented cumulative product head.

Computes, for a flat stream of samples grouped into contiguous ray segments:
    v[i]     = (1 - alpha[i]) + 1e-11
    cumprod  = segmented cumulative product of v (reset at each ray start)
    t[i]     = cumprod[i-1]  (global shift, t[0] = 1)
    weights  = alpha * t
    alphainv_last[r] = cumprod at the last sample of ray r

Device algorithm: the segmented cumprod is computed by the VectorE
`tensor_tensor_scan` instruction with ops (max, mult):
    state' = max(S[i], state) * v[i]
where S is the 0/1 segment-start mask.  At a start, max(1, state) = 1
(running products never exceed 1 + O(1e-11)), so the state resets; else
max(0, state) = state and the product continues.  The scan output is
written one column to the right, so the tile directly holds the shifted
transmittance t, and weights = alpha * t is one more elementwise op.

Sharding: the flat stream is split into 1024 ray-aligned rows
(8 cores x 128 partitions), each padded to a fixed length F, so every
partition scans an independent chain — no cross-partition or cross-core
communication.  Host numpy does the ray-aligned layout (searchsorted +
masked scatter), the start-mask encoding, the 1024 row-start weight
fixups, and the per-ray alphainv gather.
"""

import numpy as np

import concourse.bacc as bacc
import concourse.mybir as mybir
import concourse.tile as tile
from concourse.bass_utils import run_bass_kernel_spmd

# ---- problem constants (hardcoded; kernel.py must be self-contained) ----
N = 16_777_216
N_RAY = 65_536
NCORES = 8
P = 128
ROWS = NCORES * P        # 1024 ray-aligned rows
F = 16_896               # padded row length  (16384 avg + 512 slack)
NB = 8                   # column blocks per row
PAD_ALPHA = np.float32(0.5)

_f32 = mybir.dt.float32
_u8 = mybir.dt.uint8


def _block_sizes(f, nb):
    """Column-block schedule: tapered head (fast pipeline fill) and tail
    (short final serial chain) when f matches the full-size row length."""
    if f == F:
        sizes = [512, 1024] + [2560] * 5 + [1280, 640, 320, 320]
        assert sum(sizes) == f
        return sizes
    return [f // nb] * nb


def _emit_pass(tc, pools, a_ap, m_ap, w_ap, f, nb):
    """One full pass over the core's data.  a_ap [128,f] f32 alpha,
    m_ap [128,f] u8 start mask, w_ap [128,f] f32 weights out."""
    nc = tc.nc
    maskp, ld, mid, scan_pool, st = pools
    sizes = _block_sizes(f, nb)
    bmax = max(sizes)
    mult = mybir.AluOpType.mult
    amax = mybir.AluOpType.max
    Copy = mybir.ActivationFunctionType.Copy
    Identity = mybir.ActivationFunctionType.Identity

    eps = maskp.tile([P, 1], _f32, tag="eps")
    nc.gpsimd.memset(eps[:], 1e-11)
    mt = maskp.tile([P, f], _u8, tag="mask")
    nc.sync.dma_start(mt[:], m_ap[:, :])

    t_prev = None
    pw = 0
    c0 = 0
    for j, fc in enumerate(sizes):
        at = ld.tile([P, bmax], _f32, tag="alpha")
        nc.sync.dma_start(at[:, 0:fc], a_ap[:, c0 : c0 + fc])

        # u = 1 - alpha ; v = u + 1e-11   (both ScalarE)
        ut = mid.tile([P, bmax], _f32, tag="u")
        nc.scalar.activation(ut[:, 0:fc], at[:, 0:fc], Copy, bias=1.0, scale=-1.0)
        vt = mid.tile([P, bmax], _f32, tag="v")
        nc.scalar.activation(vt[:, 0:fc], ut[:, 0:fc], Identity, bias=eps[:, 0:1])

        # transmittance tile: t[:,0] is the carry-in, scan writes t[:,1:]
        tt = scan_pool.tile([P, bmax + 1], _f32, tag="t")
        if j == 0:
            nc.gpsimd.memset(tt[:, 0:1], 1.0)
        else:
            nc.scalar.copy(tt[:, 0:1], t_prev[:, pw : pw + 1])
        nc.vector.tensor_tensor_scan(
            tt[:, 1 : fc + 1],
            mt[:, c0 : c0 + fc],
            vt[:, 0:fc],
            tt[:, 0:1],
            amax,
            mult,
        )
        # w = alpha * t   (VectorE; stores go out on the scalar HWDGE ring)
        wt = st.tile([P, bmax], _f32, tag="w")
        nc.vector.tensor_tensor(wt[:, 0:fc], at[:, 0:fc], tt[:, 0:fc], mult)
        nc.scalar.dma_start(w_ap[:, c0 : c0 + fc], wt[:, 0:fc])
        t_prev = tt
        pw = fc
        c0 += fc


def build_module(f=F, nb=NB, n_cores=NCORES, loop=1):
    nc = bacc.Bacc(
        "TRN2", target_bir_lowering=False, debug=False, num_devices=n_cores
    )
    a = nc.dram_tensor("alpha", [P, f], _f32, kind="ExternalInput")
    m = nc.dram_tensor("mask", [P, f], _u8, kind="ExternalInput")
    w = nc.dram_tensor("w", [P, f], _f32, kind="ExternalOutput")
    with tile.TileContext(nc) as tc:
        with (
            tc.tile_pool(name="mask", bufs=1) as maskp,
            tc.tile_pool(name="ld", bufs=4) as ld,
            tc.tile_pool(name="mid", bufs=2) as mid,
            tc.tile_pool(name="scan", bufs=3) as scan_pool,
            tc.tile_pool(name="st", bufs=4) as st,
        ):
            pools = (maskp, ld, mid, scan_pool, st)
            if loop > 1:
                with tc.For_i(0, loop, 1):
                    _emit_pass(tc, pools, a.ap(), m.ap(), w.ap(), f, nb)
            else:
                _emit_pass(tc, pools, a.ap(), m.ap(), w.ap(), f, nb)
    nc.compile()
    return nc


_MODULE = None


def _get_module():
    global _MODULE
    if _MODULE is None:
        _MODULE = build_module()
    return _MODULE


# --------------------------- host-side helpers ---------------------------


def shard_inputs(alpha, ray_id, rows=ROWS, f=F):
    """Split the flat stream into `rows` ray-aligned rows padded to length f.

    Returns (alpha2d [rows,f] f32, mask2d [rows,f] u8, bounds [rows+1],
    lens [rows], idx [rows,f] bool).  mask2d is 1 at segment starts (every
    row start is a ray start by construction); padding columns are 0 with
    pad alpha = PAD_ALPHA so the first pad column's weight recovers the
    row's final cumprod (the "carry").
    """
    n = alpha.shape[0]
    targets = (np.arange(1, rows) * (n // rows)).astype(np.int64)
    ray_at = ray_id[targets]
    s = np.searchsorted(ray_id, ray_at, side="left")
    bounds = np.concatenate([[0], s, [n]]).astype(np.int64)
    lens = np.diff(bounds)
    assert lens.min() >= 0 and lens.max() < f, (lens.min(), lens.max(), f)

    idx = np.arange(f)[None, :] < lens[:, None]           # [rows, f]
    alpha2d = np.full((rows, f), PAD_ALPHA, np.float32)
    alpha2d[idx] = alpha
    start_flat = np.empty(n, np.uint8)
    start_flat[0] = 1
    np.not_equal(ray_id[1:], ray_id[:-1], out=start_flat[1:].view(bool))
    mask2d = np.zeros((rows, f), np.uint8)
    mask2d[idx] = start_flat
    return alpha2d, mask2d, bounds, lens, idx


def postprocess(w2d, alpha, ray_id, n_ray, bounds, lens, idx):
    """Assemble full weights + alphainv_last from the padded device output."""
    rows = w2d.shape[0]
    n = alpha.shape[0]

    # carry[k] = cumprod at the last real element of row k
    #          = t at first pad column = w[k, lens[k]] / pad_alpha
    carry = w2d[np.arange(rows), lens] * np.float32(1.0 / PAD_ALPHA)
    empty_rows = lens == 0
    if empty_rows.any():        # forward-fill carries through empty rows
        ff = np.where(empty_rows, 0, np.arange(rows))
        np.maximum.accumulate(ff, out=ff)
        carry = carry[ff]
        carry[: int(np.argmax(~empty_rows))] = 1.0

    weights = np.empty(n, np.float32)
    weights[:] = w2d[idx]

    # alphainv_last[r] = cumprod at last sample of ray r.
    cnt_hi = np.searchsorted(ray_id, np.arange(n_ray), side="right")
    cnt_lo = np.searchsorted(ray_id, np.arange(n_ray), side="left")
    nonempty = cnt_hi > cnt_lo
    last = cnt_hi - 1                                     # valid where nonempty

    alphainv = np.empty(n_ray, np.float32)
    v0 = (np.float32(1.0) - alpha[0]) + np.float32(1e-11)
    alphainv[~nonempty] = v0                              # reference quirk

    j = last[nonempty]
    row_of = np.searchsorted(bounds, j, side="right") - 1
    is_row_last = j == bounds[row_of + 1] - 1
    vals = np.empty(j.shape[0], np.float32)
    vals[is_row_last] = carry[row_of[is_row_last]]
    jn = j[~is_row_last] + 1                              # next elem, same row
    a_n = alpha[jn]
    with np.errstate(divide="ignore", invalid="ignore"):
        vals[~is_row_last] = weights[jn] / a_n
    # exact-zero (or absurdly small) alpha at the probe point: recompute ray
    bad = np.zeros(j.shape[0], bool)
    bad[~is_row_last] = a_n < 1e-20
    if bad.any():
        ray_ids_ne = np.arange(n_ray)[nonempty]
        for k in np.where(bad)[0]:
            r = ray_ids_ne[k]
            seg = alpha[cnt_lo[r] : cnt_hi[r]]
            v = (np.float32(1.0) - seg) + np.float32(1e-11)
            p = np.float32(1.0)
            for x in v:
                p = np.float32(p * x)
            vals[k] = p
    alphainv[nonempty] = vals

    # row-start weight fixups: t at a row start is the previous row's carry
    for k in range(rows):
        if lens[k] > 0:
            i = bounds[k]
            t = carry[k - 1] if k > 0 else np.float32(1.0)
            weights[i] = np.float32(alpha[i] * t)

    return weights, alphainv


def kernel(alpha, ray_id, N_ray):
    alpha = np.asarray(alpha, np.float32).reshape(-1)
    ray_id = np.asarray(ray_id, np.int32).reshape(-1)
    n_ray = int(N_ray)
    assert alpha.shape[0] == N and n_ray == N_RAY

    alpha2d, mask2d, bounds, lens, idx = shard_inputs(alpha, ray_id)
    nc = _get_module()
    in_maps = [
        {
            "alpha": np.ascontiguousarray(alpha2d[c * P : (c + 1) * P]),
            "mask": np.ascontiguousarray(mask2d[c * P : (c + 1) * P]),
        }
        for c in range(NCORES)
    ]
    res = run_bass_kernel_spmd(nc, in_maps, core_ids=list(range(NCORES)))
    w2d = np.concatenate(
        [res.results[c]["w"] for c in range(NCORES)], axis=0
    ).astype(np.float32, copy=False)
    return postprocess(w2d, alpha, ray_id, n_ray, bounds, lens, idx)
